# revision 1
# baseline (speedup 1.0000x reference)
"""SNN 5-layer conv net (nn_Net_55405078118821) for 8 Trainium2 cores. v2.

Data-parallel over batch: each core processes 4 of 32 batch elements.

Per-core dataflow (all intermediates stay in SBUF):
  - Spike/input planes stored with padded row stride ST >= win+3 so every
    conv matmul streams ONE contiguous rhs slice (no per-t segmentation):
    flat j in [0, 8*ST-3) covers 8 timesteps; garbage columns interleave
    with valid ones and are never read downstream (pad columns kept zero).
  - conv as banded bf16 matmuls, one PSUM bank per 8-row output bank.
    L1 folds all 4 column taps into K (cin=3 -> K=96) with host-prepared
    shifted x planes; precision via (wterm,xplane) product pairs.
    L2-5 run nsp weight terms x 4 taps; the 3 spill rows from the next
    input bank use tap-pair-folded spill replicas (K=96) built on-device
    by SBUF->SBUF DMA.
  - LIF scan per timestep, all on DVE with threshold-shifted state
    w = v - vth (spike iff w + dv >= 0):
      s  = (w * -1) is_le dv          (fused scalar_tensor_tensor -> bf16)
      w  = dv + w                     (tensor_tensor add)
      w  = -vth where s               (copy_predicated reset)
  - batch elements processed in interleaved pairs so the PE never waits
    on a scan tail; layer-5 spikes DMA'd out bf16; host computes means.
"""

import numpy as np
import ml_dtypes

import concourse.bass as bass
import concourse.bacc as bacc
import concourse.mybir as mybir
from concourse.tile import TileContext
from concourse.bass_utils import run_bass_kernel_spmd

N_CORES = 8
B_FULL, T = 32, 16
B_LOC = B_FULL // N_CORES
F32 = mybir.dt.float32
BF16 = mybir.dt.bfloat16
NP_BF16 = ml_dtypes.bfloat16

# (Cin, Cout, Hin, Win) per layer; Hout = Hin-3, Wout = Win-3
LAYER_SHAPES = [(3, 16, 64, 64), (16, 16, 61, 61), (16, 16, 58, 58),
                (16, 16, 55, 55), (16, 6, 52, 52)]
ST = [64, 64, 61, 58, 55]           # input row stride per layer
OST = [64, 61, 58, 55, 49]          # output tile row stride per layer
# precision: L1 (wterm, xplane) product pairs; L2-5 bf16 weight terms
L1_PAIRS = [(0, 0), (0, 1), (1, 0)]
NP1 = len(L1_PAIRS)
NSP = [None, 2, 2, 2, 1]


class LayerCfg:
    def __init__(self, idx, cin, cout, hin, win):
        self.idx = idx
        self.cin, self.cout, self.hin, self.win = cin, cout, hin, win
        self.hout, self.wout = hin - 3, win - 3
        self.nbk_in = (hin + 7) // 8
        self.nbk_out = (self.hout + 7) // 8
        self.st = ST[idx]
        self.ost = OST[idx]
        self.n = 8 * self.st - 3                     # matmul moving size
        self.mf = 8 * cout                           # full-block M
        self.p = min(self.mf, 128)
        self.nsp = NSP[idx]
        self.banks = []
        for q in range(self.nbk_out):
            r = min(8, self.hout - 8 * q)            # valid out rows
            r1 = min(8, hin - 8 * q)                 # in rows in bank q
            r2 = max(0, r - 5)                       # spill rows used
            self.banks.append((q, r, r1, r2))
        self.groups = [list(range(0, min(4, self.nbk_out))),
                       list(range(4, self.nbk_out))]


CFGS = [LayerCfg(i, *s) for i, s in enumerate(LAYER_SHAPES)]
L5 = CFGS[-1]
SOUT_FREE = L5.nbk_out * T * L5.ost                  # 7*16*49 = 5488


def _bf16_terms(a, n):
    a = np.asarray(a, np.float32)
    terms = []
    for _ in range(n):
        t = a.astype(NP_BF16).astype(np.float32)
        terms.append(t)
        a = a - t
    return terms


def _pack_A1(w):
    """L1 stationary: K=(rm,ci,dj)=96, M=(rho,co)=128, dj folded into K."""
    a = np.zeros((96, 128), np.float32)
    for rm in range(8):
        for ci in range(3):
            for dj in range(4):
                k = rm * 12 + ci * 4 + dj
                for rho in range(max(0, rm - 3), rm + 1):
                    a[k, rho * 16:(rho + 1) * 16] = w[:, ci, rm - rho, dj]
    return a


def _pack_B1(w):
    """L1 spill: K=(rm 0..2,ci,dj)=36, input row = 8+rm."""
    b = np.zeros((36, 128), np.float32)
    for rm in range(3):
        for ci in range(3):
            for dj in range(4):
                k = rm * 12 + ci * 4 + dj
                for rho in range(rm + 5, 8):
                    di = rm + 8 - rho
                    if 0 <= di <= 3:
                        b[k, rho * 16:(rho + 1) * 16] = w[:, ci, di, dj]
    return b


def _pack_A(w):
    """L2-5 stationary per term: K=(rm,ci)=8*cin, M=(rho,co), banded."""
    cout, cin = w.shape[0], w.shape[1]
    mf = 8 * cout
    a = np.zeros((8 * cin, 4 * mf), np.float32)
    for dj in range(4):
        for rm in range(8):
            for rho in range(max(0, rm - 3), rm + 1):
                a[rm * cin:(rm + 1) * cin,
                  dj * mf + rho * cout: dj * mf + (rho + 1) * cout] = \
                    w[:, :, rm - rho, dj].T
    return a


def _pack_B(w, rmax=3):
    """L2-5 spill, tap-pair folded: K=(tap,rm,ci)=6*cin, two tap groups.
    rmax<3 zeroes spill rows rm>=rmax (for banks whose last input rows
    don't exist)."""
    cout, cin = w.shape[0], w.shape[1]
    mf = 8 * cout
    b = np.zeros((6 * cin, 2 * mf), np.float32)
    for g in range(2):
        for tap in range(2):
            for rm in range(rmax):
                dj = 2 * g + tap
                k0 = (tap * 3 + rm) * cin
                for rho in range(rm + 5, 8):
                    di = rm + 8 - rho
                    if 0 <= di <= 3:
                        b[k0:k0 + cin,
                          g * mf + rho * cout: g * mf + (rho + 1) * cout] = \
                            w[:, :, di, dj].T
    return b


def _pack_weights(inputs):
    m = {}
    terms = _bf16_terms(np.asarray(inputs["w1"], np.float32),
                        1 + max(wi for wi, _ in L1_PAIRS))
    m["wA1"] = np.concatenate(
        [_pack_A1(terms[wi]) for wi, _ in L1_PAIRS], axis=1).astype(NP_BF16)
    m["wB1"] = np.concatenate(
        [_pack_B1(terms[wi]) for wi, _ in L1_PAIRS], axis=1).astype(NP_BF16)
    for li in range(1, 5):
        cfg = CFGS[li]
        w = np.asarray(inputs[f"w{li + 1}"], np.float32)
        terms = _bf16_terms(w, cfg.nsp)
        m[f"wA{li + 1}"] = np.concatenate(
            [_pack_A(-t) for t in terms], axis=1).astype(NP_BF16)
        m[f"wB{li + 1}"] = np.concatenate(
            [_pack_B(-t) for t in terms], axis=1).astype(NP_BF16)
        if li == 2:
            # zero-padded variant for the partial-spill bank (L3 q=6)
            m[f"wBp{li + 1}"] = np.concatenate(
                [_pack_B(-t, rmax=2) for t in terms], axis=1).astype(NP_BF16)
    return m


def _pack_scalars(inputs):
    """Per-partition per-layer scalars: vthp = vth, cc = C, where C[co]
    is the quantized-weight kernel sum (conv(ones)); C=0 for layer 1."""
    s1 = np.zeros((128, 5), np.float32)
    s2 = np.zeros((128, 5), np.float32)
    for li, cfg in enumerate(CFGS):
        v = np.asarray(inputs[f"vth{li + 1}"], np.float32).reshape(-1)
        if li == 0:
            c = np.zeros(cfg.cout, np.float32)
        else:
            w = np.asarray(inputs[f"w{li + 1}"], np.float32)
            terms = _bf16_terms(w, cfg.nsp)
            c = sum(t.sum(axis=(1, 2, 3)) for t in terms).astype(np.float32)
        for p in range(cfg.p):
            s1[p, li] = v[p % cfg.cout]
            s2[p, li] = -c[p % cfg.cout]
    return s1, s2


def _arrange_x(x):
    """[b,T,3,64,64] -> bf16 [b, 2, 96=(rm*12+ci*4+dj), (q*16+t)*64+w],
    value = xplane[b,t,ci,8q+rm,w+dj] (zero beyond column 63)."""
    bl = x.shape[0]
    planes = _bf16_terms(x, 2)
    out = np.zeros((bl, 2, 8, 3, 4, 8, T, 64), NP_BF16)  # b pl rm ci dj q t w
    for pl in range(2):
        src = planes[pl].reshape(bl, T, 3, 8, 8, 64)     # b t ci q rm w
        src = src.transpose(0, 4, 2, 3, 1, 5)            # b rm ci q t w
        for dj in range(4):
            out[:, pl, :, :, dj, :, :, :64 - dj] = src[..., dj:]
    out = out.reshape(bl, 2, 96, 8 * T * 64)
    return np.ascontiguousarray(out)


_PROGRAM_CACHE = {}


def _build_program():
    if "nc" in _PROGRAM_CACHE:
        return _PROGRAM_CACHE["nc"]
    nc = bacc.Bacc("TRN2", target_bir_lowering=False, debug=False)

    x_d = nc.dram_tensor("xr", [B_LOC, 2, 96, 8 * T * 64], BF16,
                         kind="ExternalInput").ap()
    wa_d = {0: nc.dram_tensor("wA1", [96, NP1 * 128], BF16,
                              kind="ExternalInput").ap()}
    wb_d = {0: nc.dram_tensor("wB1", [36, NP1 * 128], BF16,
                              kind="ExternalInput").ap()}
    wbp_d = {}
    for li in range(1, 5):
        cfg = CFGS[li]
        wa_d[li] = nc.dram_tensor(f"wA{li + 1}",
                                  [128, cfg.nsp * 4 * cfg.mf], BF16,
                                  kind="ExternalInput").ap()
        wb_d[li] = nc.dram_tensor(f"wB{li + 1}",
                                  [96, cfg.nsp * 2 * cfg.mf], BF16,
                                  kind="ExternalInput").ap()
        if li == 2:
            wbp_d[li] = nc.dram_tensor(f"wBp{li + 1}",
                                       [96, cfg.nsp * 2 * cfg.mf], BF16,
                                       kind="ExternalInput").ap()
    vthp_d = nc.dram_tensor("vthp", [128, 5], F32,
                            kind="ExternalInput").ap()
    cc_d = nc.dram_tensor("cc", [128, 5], F32,
                          kind="ExternalInput").ap()
    sout_d = nc.dram_tensor("sout", [B_LOC, 48, SOUT_FREE], BF16,
                            kind="ExternalOutput").ap()

    with TileContext(nc) as tc:
        with (
            tc.tile_pool(name="wts", bufs=1) as wts,
            tc.tile_pool(name="xin", bufs=1) as xpool,
            tc.tile_pool(name="spk", bufs=1) as spool,
            tc.tile_pool(name="spill", bufs=1) as fpool,
            tc.tile_pool(name="scan", bufs=1) as upool,
            tc.tile_pool(name="ev", bufs=2) as epool,
            tc.tile_pool(name="psum", bufs=2, space="PSUM") as ppool,
        ):
            # --- constants ---
            wa_t, wb_t, wbp_t = {}, {}, {}
            wa_t[0] = wts.tile([96, NP1 * 128], BF16, tag="wa0", name="wa0")
            nc.sync.dma_start(out=wa_t[0][:, :], in_=wa_d[0])
            wb_t[0] = wts.tile([36, NP1 * 128], BF16, tag="wb0", name="wb0")
            nc.sync.dma_start(out=wb_t[0][:, :], in_=wb_d[0])
            for li in range(1, 5):
                cfg = CFGS[li]
                wa_t[li] = wts.tile([128, cfg.nsp * 4 * cfg.mf], BF16,
                                    tag=f"wa{li}", name=f"wa{li}")
                nc.sync.dma_start(out=wa_t[li][:, :], in_=wa_d[li])
                wb_t[li] = wts.tile([96, cfg.nsp * 2 * cfg.mf], BF16,
                                    tag=f"wb{li}", name=f"wb{li}")
                nc.sync.dma_start(out=wb_t[li][:, :], in_=wb_d[li])
                if li == 2:
                    wbp_t[li] = wts.tile([96, cfg.nsp * 2 * cfg.mf], BF16,
                                         tag=f"wbp{li}", name=f"wbp{li}")
                    nc.sync.dma_start(out=wbp_t[li][:, :], in_=wbp_d[li])
            vthp_t = wts.tile([128, 5], F32, tag="vthp")
            nc.sync.dma_start(out=vthp_t[:, :], in_=vthp_d)
            cc_t = wts.tile([128, 5], F32, tag="cc")
            nc.sync.dma_start(out=cc_t[:, :], in_=cc_d)
            zero_t = wts.tile([128, 512], F32, tag="zero")
            nc.vector.memset(zero_t[:, :], 0.0)

            # spike tiles [128, 8192] tags (b%2, li%2); spill [96, 7168]
            # tags (b%2, tapgrp); x [96, 8192] tags (b%2, plane);
            # w-state [128, 256] f32 tags (b%2, group)
            s_t, f_t, x_t, w_t = {}, {}, {}, {}
            for pl in range(2):
                x_t[pl] = xpool.tile([96, 8192], BF16,
                                     tag=f"x{pl}", name=f"x{pl}")
            for bp in range(2):
                for lp in range(2):
                    s_t[(bp, lp)] = spool.tile([128, 8192], BF16,
                                               tag=f"s{bp}{lp}",
                                               name=f"s{bp}{lp}")
                for g in range(2):
                    f_t[(bp, g)] = fpool.tile([96, 7 * T * 64], BF16,
                                              tag=f"f{bp}{g}",
                                              name=f"f{bp}{g}")

                w_t[bp] = upool.tile(
                    [128, 512], F32, tag=f"w{bp}", name=f"w{bp}")

            def emit_x_dma(b):
                for pl in range(2):
                    nc.sync.dma_start(out=x_t[pl][:, :], in_=x_d[b, pl])

            def emit_spill(b, li):
                """Spill replicas for layer li>=1 from layer li-1 spikes:
                f[(tap*3+rm)*16+ci, q, t, w] = s[rm*16+ci, q+1, t, w+2g+tap].
                One contiguous 48-partition DMA per (g, tap)."""
                bp = b % 2
                cfg = CFGS[li]
                st, nq = cfg.st, cfg.nbk_in - 1
                src_t = s_t[(bp, (li - 1) % 2)]
                src_v = src_t[:, 0:CFGS[li - 1].nbk_out * T * st].rearrange(
                    "p (q t w) -> p q t w", t=T, w=st)
                for g in range(2):
                    dst_t = f_t[(bp, g)]
                    dst_v = dst_t[:, 0:nq * T * st].rearrange(
                        "p (q t w) -> p q t w", t=T, w=st)
                    for tap in range(2):
                        sh = 2 * g + tap
                        dst = dst_v[tap * 48:tap * 48 + 48, :, :, 0:st - sh]
                        src = src_v[0:48, 1:nq + 1, :, sh:st]
                        nc.gpsimd.dma_start(out=dst, in_=src)

            def emit_layer(b, li):
                bp = b % 2
                cfg = CFGS[li]
                p, mf, n, st, ost = cfg.p, cfg.mf, cfg.n, cfg.st, cfg.ost
                wout = cfg.wout
                ipl = 16 * st                          # input q-plane size
                s_in = None if li == 0 else s_t[(bp, (li - 1) % 2)]
                s_out = s_t[(bp, li % 2)]
                sov = s_out[:, 0:cfg.nbk_out * T * ost].rearrange(
                    "p (q t w) -> p q t w", t=T, w=ost)
                vth_ap = vthp_t[0:p, li:li + 1]
                c_ap = cc_t[0:p, li:li + 1]

                for h in range(2):
                    base_h = h * 8 * st
                    et = epool.tile([128, 3904], F32, tag="e",
                                    name=f"e{b}l{li}h{h}")
                    ev4 = et[0:p, 0:cfg.nbk_out * 8 * wout].rearrange(
                        "p (k t w) -> p k t w", t=8, w=wout)
                    for gi, qs in enumerate(cfg.groups):
                        nbkg = len(qs)
                        ps = ppool.tile([128, 2048], F32, tag="ps",
                                        name=f"ps{b}l{li}h{h}g{gi}")
                        ps_f = ps[:, :]
                        nmm = {}
                        for bi, q in enumerate(qs):
                            _, r, r1, r2 = cfg.banks[q]
                            na = NP1 if li == 0 else cfg.nsp * 4
                            nb = (NP1 if li == 0 else cfg.nsp * 2) \
                                if r2 > 0 else 0
                            nmm[bi] = [na + nb, 0]

                        def mm(bi, lhs, rhs):
                            tot, done = nmm[bi]
                            out_ap = ps_f[0:p, bi * 512: bi * 512 + n]
                            nc.tensor.matmul(out_ap, lhs, rhs,
                                             start=(done == 0),
                                             stop=(done == tot - 1))
                            nmm[bi][1] += 1

                        if li == 0:
                            for pi, (wi, xi) in enumerate(L1_PAIRS):
                                lhs = wa_t[0][0:96, pi * 128:(pi + 1) * 128]
                                xt = x_t[xi][:, :]
                                for bi, q in enumerate(qs):
                                    rhs = xt[0:96, q * 1024 + base_h:
                                             q * 1024 + base_h + n]
                                    mm(bi, lhs, rhs)
                            for pi, (wi, xi) in enumerate(L1_PAIRS):
                                lhs = wb_t[0][0:36, pi * 128:(pi + 1) * 128]
                                xt = x_t[xi][:, :]
                                for bi, q in enumerate(qs):
                                    if cfg.banks[q][3] > 0:
                                        rhs = xt[0:36,
                                                 (q + 1) * 1024 + base_h:
                                                 (q + 1) * 1024 + base_h + n]
                                        mm(bi, lhs, rhs)
                        else:
                            s_in_f = s_in[:, :]
                            for sp in range(cfg.nsp):
                                for dj in range(4):
                                    c0 = (sp * 4 + dj) * mf
                                    for bi, q in enumerate(qs):
                                        k1 = cfg.banks[q][2] * 16
                                        lhs = wa_t[li][0:k1, c0:c0 + mf]
                                        rhs = s_in_f[0:k1,
                                                     q * ipl + base_h + dj:
                                                     q * ipl + base_h + dj + n]
                                        mm(bi, lhs, rhs)
                            for sp in range(cfg.nsp):
                                for g in range(2):
                                    c0 = (sp * 2 + g) * mf
                                    ft = f_t[(bp, g)][:, :]
                                    for bi, q in enumerate(qs):
                                        r2 = cfg.banks[q][3]
                                        if r2 > 0:
                                            wbt = wb_t[li] if r2 == 3 \
                                                else wbp_t[li]
                                            lhs = wbt[0:96, c0:c0 + mf]
                                            rhs = ft[0:96,
                                                     q * ipl + base_h:
                                                     q * ipl + base_h + n]
                                            mm(bi, lhs, rhs)

                        # --- evict psum to SBUF on the idle ACT engine,
                        # negated and with the conv(ones) constant folded:
                        # e = -(q + C) = -dv. Frees the psum bank after one
                        # instruction; all groups land in one e tile. ---
                        ps_v = ps_f.rearrange("p (k f) -> p k f", k=4)
                        ps4 = ps_v[0:p, 0:nbkg, 0:8 * st].rearrange(
                            "p k (t w) -> p k t w", w=st)[:, :, :, 0:wout]
                        epart = ev4[0:p, qs[0]:qs[0] + nbkg, :, :]
                        nc.scalar.activation(
                            epart, ps4, mybir.ActivationFunctionType.Identity,
                            bias=c_ap, scale=-1.0)

                    # --- LIF scan over all banks (SBUF only, all DVE);
                    # e = -dv so the spike test fits one fused op; z tiles:
                    #   z  = (v_prev - vth) is_lt e    [v' < vth]
                    #   v' = v_prev - e
                    #   v  = z * v'
                    nbt = cfg.nbk_out
                    vv = w_t[bp][:, :].rearrange(
                        "p (k w) -> p k w", w=64)[0:p, 0:nbt, 0:wout]
                    zv = zero_t[:, :].rearrange(
                        "p (k w) -> p k w", w=64)[0:p, 0:nbt, 0:wout]
                    for t in range(8):
                        tt = h * 8 + t
                        ev = ev4[0:p, 0:nbt, t, 0:wout]
                        v_prev = zv if tt == 0 else vv
                        sw = sov[0:p, 0:nbt, tt, 0:wout]
                        nc.vector.scalar_tensor_tensor(
                            out=sw, in0=v_prev, scalar=vth_ap,
                            in1=ev, op0=mybir.AluOpType.subtract,
                            op1=mybir.AluOpType.is_lt)
                        nc.vector.tensor_tensor(
                            out=vv, in0=v_prev, in1=ev,
                            op=mybir.AluOpType.subtract)
                        nc.vector.tensor_tensor(
                            out=vv, in0=sw, in1=vv,
                            op=mybir.AluOpType.mult)

            for pair in ((0, 1), (2, 3)):
                for li in range(5):
                    for b in pair:
                        if li == 0:
                            emit_x_dma(b)
                        emit_layer(b, li)
                        if li < 4:
                            emit_spill(b, li + 1)
                for b in pair:
                    nc.gpsimd.dma_start(
                        out=sout_d[b],
                        in_=s_t[(b % 2, 0)][0:48, 0:SOUT_FREE])

    nc.compile()
    _PROGRAM_CACHE["nc"] = nc
    return nc


def _host_inputs(inputs):
    m = _pack_weights(inputs)
    s1, s2 = _pack_scalars(inputs)
    m["vthp"] = s1
    m["cc"] = s2
    return m


def decode_sout(sout):
    """[B_LOC, 48, SOUT_FREE] bf16 -> [B_LOC, T, 6] spike means."""
    a = 1.0 - np.asarray(sout, np.float32).reshape(B_LOC, 8, 6, L5.nbk_out,
                                                   T, L5.ost)
    rho = np.arange(8)[:, None]
    qq = np.arange(L5.nbk_out)[None, :]
    mask = (8 * qq + rho) < L5.hout                     # [rho, q]
    a = a.transpose(0, 4, 2, 1, 3, 5)                   # [b, t, c, rho, q, j]
    vals = a[:, :, :, mask, :][:, :, :, :, :L5.wout]    # [b, t, c, 49, 49]
    return vals.mean(axis=(3, 4)).astype(np.float32)


def run_spmd(inputs, **kw):
    nc = _build_program()
    x = np.asarray(inputs["x"], np.float32)
    const = _host_inputs(inputs)
    in_maps = []
    for c in range(N_CORES):
        m = dict(const)
        m["xr"] = _arrange_x(x[c * B_LOC:(c + 1) * B_LOC])
        in_maps.append(m)
    return run_bass_kernel_spmd(nc, in_maps, list(range(N_CORES)), **kw)


def kernel(**inputs):
    res = run_spmd(inputs)
    outs = [decode_sout(r["sout"]) for r in res.results]
    return np.concatenate(outs, axis=0)



# revision 4
# speedup vs baseline: 1.1668x; 1.1668x over previous
"""SNN 5-layer conv net (nn_Net_55405078118821) for 8 Trainium2 cores. v3.

Data-parallel over batch: each core processes 4 of 32 batch elements.

Per-core dataflow (all intermediates stay in SBUF):
  - Spike/input planes stored with padded EVEN row stride ST >= win+3 so
    every conv matmul streams ONE contiguous rhs slice (no per-t
    segmentation) and every scan/eviction op has even, 4B-aligned runs
    (enables DVE 2x perf modes). Garbage pad columns are never read by
    valid outputs.
  - conv as banded bf16 matmuls, one PSUM bank per 8-row output bank.
    L1 folds all 4 column taps into K (cin=3 -> K=96) with host-prepared
    shifted x planes; precision via (wterm,xplane) product pairs.
    L2-5 run nsp weight terms x 4 taps; the 3 spill rows from the next
    input bank use tap-pair-folded spill replicas (K=96) built on-device
    by SBUF->SBUF DMA.
  - LIF scan per timestep on DVE (f32 state, inverted spikes s'=1-s):
      u  = v - e            (tensor_tensor; e = -dv)
      s' = u is_lt vth      (tensor_scalar single-src -> 2x mode, bf16)
      v  = u * s'           (tensor_tensor reset)
  - Rolling batch schedule (no pair barrier): batches pipeline two-deep
    through the layers; next batch's L1 slots right after the previous
    pair's L4 so its x DMA and the L5 tails overlap; layer-5 spikes
    DMA'd out bf16; host computes means.
"""

import numpy as np
import ml_dtypes

import concourse.bass as bass
import concourse.bacc as bacc
import concourse.mybir as mybir
from concourse.tile import TileContext
from concourse.bass_utils import run_bass_kernel_spmd

N_CORES = 8
B_FULL, T = 32, 16
B_LOC = B_FULL // N_CORES
F32 = mybir.dt.float32
BF16 = mybir.dt.bfloat16
NP_BF16 = ml_dtypes.bfloat16

# (Cin, Cout, Hin, Win) per layer; Hout = Hin-3, Wout = Win-3
LAYER_SHAPES = [(3, 16, 64, 64), (16, 16, 61, 61), (16, 16, 58, 58),
                (16, 16, 55, 55), (16, 6, 52, 52)]
ST = [64, 62, 58, 56, 52]           # input row stride per layer (even)
OST = [62, 58, 56, 52, 50]          # output tile row stride per layer (even)
# precision: L1 (wterm, xplane) product pairs; L2-5 bf16 weight terms
L1_PAIRS = [(0, 0), (0, 1), (1, 0)]
NP1 = len(L1_PAIRS)
NSP = [None, 2, 2, 2, 1]


class LayerCfg:
    def __init__(self, idx, cin, cout, hin, win):
        self.idx = idx
        self.cin, self.cout, self.hin, self.win = cin, cout, hin, win
        self.hout, self.wout = hin - 3, win - 3
        self.nbk_in = (hin + 7) // 8
        self.nbk_out = (self.hout + 7) // 8
        self.st = ST[idx]
        self.ost = OST[idx]
        self.n = 8 * self.st - 3                     # matmul moving size
        self.mf = 8 * cout                           # full-block M
        self.p = min(self.mf, 128)
        self.nsp = NSP[idx]
        self.banks = []
        for q in range(self.nbk_out):
            r = min(8, self.hout - 8 * q)            # valid out rows
            r1 = min(8, hin - 8 * q)                 # in rows in bank q
            r2 = max(0, r - 5)                       # spill rows used
            self.banks.append((q, r, r1, r2))
        self.groups = [list(range(0, min(4, self.nbk_out))),
                       list(range(4, self.nbk_out))]


CFGS = [LayerCfg(i, *s) for i, s in enumerate(LAYER_SHAPES)]
L5 = CFGS[-1]
SOUT_FREE = L5.nbk_out * T * L5.ost                  # 7*16*50 = 5600


def _bf16_terms(a, n):
    a = np.asarray(a, np.float32)
    terms = []
    for _ in range(n):
        t = a.astype(NP_BF16).astype(np.float32)
        terms.append(t)
        a = a - t
    return terms


def _pack_A1(w):
    """L1 stationary: K=(rm,ci,dj)=96, M=(rho,co)=128, dj folded into K."""
    a = np.zeros((96, 128), np.float32)
    for rm in range(8):
        for ci in range(3):
            for dj in range(4):
                k = rm * 12 + ci * 4 + dj
                for rho in range(max(0, rm - 3), rm + 1):
                    a[k, rho * 16:(rho + 1) * 16] = w[:, ci, rm - rho, dj]
    return a


def _pack_B1(w):
    """L1 spill: K=(rm 0..2,ci,dj)=36, input row = 8+rm."""
    b = np.zeros((36, 128), np.float32)
    for rm in range(3):
        for ci in range(3):
            for dj in range(4):
                k = rm * 12 + ci * 4 + dj
                for rho in range(rm + 5, 8):
                    di = rm + 8 - rho
                    if 0 <= di <= 3:
                        b[k, rho * 16:(rho + 1) * 16] = w[:, ci, di, dj]
    return b


def _pack_A(w):
    """L2-5 stationary per term: K=(rm,ci)=8*cin, M=(rho,co), banded."""
    cout, cin = w.shape[0], w.shape[1]
    mf = 8 * cout
    a = np.zeros((8 * cin, 4 * mf), np.float32)
    for dj in range(4):
        for rm in range(8):
            for rho in range(max(0, rm - 3), rm + 1):
                a[rm * cin:(rm + 1) * cin,
                  dj * mf + rho * cout: dj * mf + (rho + 1) * cout] = \
                    w[:, :, rm - rho, dj].T
    return a


def _pack_B(w, rmax=3):
    """L2-5 spill, tap-pair folded: K=(tap,rm,ci)=6*cin, two tap groups.
    rmax<3 zeroes spill rows rm>=rmax (for banks whose last input rows
    don't exist)."""
    cout, cin = w.shape[0], w.shape[1]
    mf = 8 * cout
    b = np.zeros((6 * cin, 2 * mf), np.float32)
    for g in range(2):
        for tap in range(2):
            for rm in range(rmax):
                dj = 2 * g + tap
                k0 = (tap * 3 + rm) * cin
                for rho in range(rm + 5, 8):
                    di = rm + 8 - rho
                    if 0 <= di <= 3:
                        b[k0:k0 + cin,
                          g * mf + rho * cout: g * mf + (rho + 1) * cout] = \
                            w[:, :, di, dj].T
    return b


def _pack_weights(inputs):
    m = {}
    terms = _bf16_terms(np.asarray(inputs["w1"], np.float32),
                        1 + max(wi for wi, _ in L1_PAIRS))
    m["wA1"] = np.concatenate(
        [_pack_A1(terms[wi]) for wi, _ in L1_PAIRS], axis=1).astype(NP_BF16)
    m["wB1"] = np.concatenate(
        [_pack_B1(terms[wi]) for wi, _ in L1_PAIRS], axis=1).astype(NP_BF16)
    for li in range(1, 5):
        cfg = CFGS[li]
        w = np.asarray(inputs[f"w{li + 1}"], np.float32)
        terms = _bf16_terms(w, cfg.nsp)
        m[f"wA{li + 1}"] = np.concatenate(
            [_pack_A(-t) for t in terms], axis=1).astype(NP_BF16)
        m[f"wB{li + 1}"] = np.concatenate(
            [_pack_B(-t) for t in terms], axis=1).astype(NP_BF16)
        if li == 2:
            # zero-padded variant for the partial-spill bank (L3 q=6)
            m[f"wBp{li + 1}"] = np.concatenate(
                [_pack_B(-t, rmax=2) for t in terms], axis=1).astype(NP_BF16)
    return m


def _pack_scalars(inputs):
    """Per-partition per-layer scalars: vthp = vth, cc = C, where C[co]
    is the quantized-weight kernel sum (conv(ones)); C=0 for layer 1."""
    s1 = np.zeros((128, 5), np.float32)
    s2 = np.zeros((128, 5), np.float32)
    for li, cfg in enumerate(CFGS):
        v = np.asarray(inputs[f"vth{li + 1}"], np.float32).reshape(-1)
        if li == 0:
            c = np.zeros(cfg.cout, np.float32)
        else:
            w = np.asarray(inputs[f"w{li + 1}"], np.float32)
            terms = _bf16_terms(w, cfg.nsp)
            c = sum(t.sum(axis=(1, 2, 3)) for t in terms).astype(np.float32)
        for p in range(cfg.p):
            s1[p, li] = v[p % cfg.cout]
            s2[p, li] = -c[p % cfg.cout]
    return s1, s2


def _arrange_x(x):
    """[b,T,3,64,64] -> bf16 [b, 2, 96=(rm*12+ci*4+dj), (q*16+t)*64+w],
    value = xplane[b,t,ci,8q+rm,w+dj] (zero beyond column 63)."""
    bl = x.shape[0]
    planes = _bf16_terms(x, 2)
    out = np.zeros((bl, 2, 8, 3, 4, 8, T, 64), NP_BF16)  # b pl rm ci dj q t w
    for pl in range(2):
        src = planes[pl].reshape(bl, T, 3, 8, 8, 64)     # b t ci q rm w
        src = src.transpose(0, 4, 2, 3, 1, 5)            # b rm ci q t w
        for dj in range(4):
            out[:, pl, :, :, dj, :, :, :64 - dj] = src[..., dj:]
    out = out.reshape(bl, 2, 96, 8 * T * 64)
    return np.ascontiguousarray(out)


_PROGRAM_CACHE = {}

# rolling slot order (b, li): two batches pipeline per layer; the next
# pair's L1 slots interleave with the previous pair's L5 so PE never
# drains while x DMA / scan tails complete.
SLOT_SEQ = [(0, 0), (1, 0), (0, 1), (1, 1), (0, 2), (1, 2), (0, 3), (1, 3),
            (0, 4), (2, 0), (1, 4), (3, 0), (2, 1), (3, 1), (2, 2), (3, 2),
            (2, 3), (3, 3), (2, 4), (3, 4)]


def _build_program():
    if "nc" in _PROGRAM_CACHE:
        return _PROGRAM_CACHE["nc"]
    nc = bacc.Bacc("TRN2", target_bir_lowering=False, debug=False)

    x_d = nc.dram_tensor("xr", [B_LOC, 2, 96, 8 * T * 64], BF16,
                         kind="ExternalInput").ap()
    wa_d = {0: nc.dram_tensor("wA1", [96, NP1 * 128], BF16,
                              kind="ExternalInput").ap()}
    wb_d = {0: nc.dram_tensor("wB1", [36, NP1 * 128], BF16,
                              kind="ExternalInput").ap()}
    wbp_d = {}
    for li in range(1, 5):
        cfg = CFGS[li]
        wa_d[li] = nc.dram_tensor(f"wA{li + 1}",
                                  [128, cfg.nsp * 4 * cfg.mf], BF16,
                                  kind="ExternalInput").ap()
        wb_d[li] = nc.dram_tensor(f"wB{li + 1}",
                                  [96, cfg.nsp * 2 * cfg.mf], BF16,
                                  kind="ExternalInput").ap()
        if li == 2:
            wbp_d[li] = nc.dram_tensor(f"wBp{li + 1}",
                                       [96, cfg.nsp * 2 * cfg.mf], BF16,
                                       kind="ExternalInput").ap()
    vthp_d = nc.dram_tensor("vthp", [128, 5], F32,
                            kind="ExternalInput").ap()
    cc_d = nc.dram_tensor("cc", [128, 5], F32,
                          kind="ExternalInput").ap()
    sout_d = nc.dram_tensor("sout", [B_LOC, 48, SOUT_FREE], BF16,
                            kind="ExternalOutput").ap()

    s_size = {0: max(CFGS[0].nbk_out * T * CFGS[0].ost,
                     CFGS[2].nbk_out * T * CFGS[2].ost,
                     CFGS[4].nbk_out * T * CFGS[4].ost),
              1: max(CFGS[1].nbk_out * T * CFGS[1].ost,
                     CFGS[3].nbk_out * T * CFGS[3].ost)}
    f_size = max((CFGS[li].nbk_in - 1) * T * CFGS[li].st
                 for li in range(1, 5))
    e_size = max(c.nbk_out * 8 * c.ost for c in CFGS)

    with TileContext(nc) as tc:
        with (
            tc.tile_pool(name="wts", bufs=1) as wts,
            tc.tile_pool(name="xin", bufs=1) as xpool,
            tc.tile_pool(name="spk", bufs=1) as spool,
            tc.tile_pool(name="spill", bufs=1) as fpool,
            tc.tile_pool(name="scan", bufs=1) as upool,
            tc.tile_pool(name="ev", bufs=2) as epool,
            tc.tile_pool(name="psum", bufs=2, space="PSUM") as ppool,
        ):
            # --- constants ---
            wa_t, wb_t, wbp_t = {}, {}, {}
            wa_t[0] = wts.tile([96, NP1 * 128], BF16, tag="wa0", name="wa0")
            nc.sync.dma_start(out=wa_t[0][:, :], in_=wa_d[0])
            wb_t[0] = wts.tile([36, NP1 * 128], BF16, tag="wb0", name="wb0")
            nc.sync.dma_start(out=wb_t[0][:, :], in_=wb_d[0])
            for li in range(1, 5):
                cfg = CFGS[li]
                wa_t[li] = wts.tile([128, cfg.nsp * 4 * cfg.mf], BF16,
                                    tag=f"wa{li}", name=f"wa{li}")
                nc.sync.dma_start(out=wa_t[li][:, :], in_=wa_d[li])
                wb_t[li] = wts.tile([96, cfg.nsp * 2 * cfg.mf], BF16,
                                    tag=f"wb{li}", name=f"wb{li}")
                nc.sync.dma_start(out=wb_t[li][:, :], in_=wb_d[li])
                if li == 2:
                    wbp_t[li] = wts.tile([96, cfg.nsp * 2 * cfg.mf], BF16,
                                         tag=f"wbp{li}", name=f"wbp{li}")
                    nc.sync.dma_start(out=wbp_t[li][:, :], in_=wbp_d[li])
            vthp_t = wts.tile([128, 5], F32, tag="vthp")
            nc.sync.dma_start(out=vthp_t[:, :], in_=vthp_d)
            cc_t = wts.tile([128, 5], F32, tag="cc")
            nc.sync.dma_start(out=cc_t[:, :], in_=cc_d)

            # spike tiles tags (b%2, li%2); spill [96, f_size] tags
            # (b%2, tapgrp); x [96, 8192] tags plane; v/u state [128, 512]
            # f32 tags (b%2)
            s_t, f_t, x_t, w_t, u_t = {}, {}, {}, {}, {}
            for pl in range(2):
                x_t[pl] = xpool.tile([96, 8192], BF16,
                                     tag=f"x{pl}", name=f"x{pl}")
            for bp in range(2):
                for lp in range(2):
                    s_t[(bp, lp)] = spool.tile([128, s_size[lp]], BF16,
                                               tag=f"s{bp}{lp}",
                                               name=f"s{bp}{lp}")
                for g in range(2):
                    f_t[(bp, g)] = fpool.tile([96, f_size], BF16,
                                              tag=f"f{bp}{g}",
                                              name=f"f{bp}{g}")
                w_t[bp] = upool.tile(
                    [128, 512], F32, tag=f"w{bp}", name=f"w{bp}")
                u_t[bp] = upool.tile(
                    [128, 512], F32, tag=f"u{bp}", name=f"u{bp}")

            def emit_x_dma(b):
                for pl in range(2):
                    nc.sync.dma_start(out=x_t[pl][:, :], in_=x_d[b, pl])

            def emit_spill(b, li):
                """Spill replicas for layer li>=1 from layer li-1 spikes:
                f[(tap*3+rm)*16+ci, q, t, w] = s[rm*16+ci, q+1, t, w+2g+tap].
                One contiguous 48-partition DMA per (g, tap)."""
                bp = b % 2
                cfg = CFGS[li]
                st, nq = cfg.st, cfg.nbk_in - 1
                src_t = s_t[(bp, (li - 1) % 2)]
                src_v = src_t[:, 0:CFGS[li - 1].nbk_out * T * st].rearrange(
                    "p (q t w) -> p q t w", t=T, w=st)
                for g in range(2):
                    dst_t = f_t[(bp, g)]
                    dst_v = dst_t[:, 0:nq * T * st].rearrange(
                        "p (q t w) -> p q t w", t=T, w=st)
                    for tap in range(2):
                        sh = 2 * g + tap
                        dst = dst_v[tap * 48:tap * 48 + 48, :, :, 0:st - sh]
                        src = src_v[0:48, 1:nq + 1, :, sh:st]
                        nc.gpsimd.dma_start(out=dst, in_=src)

            def emit_layer(b, li):
                bp = b % 2
                cfg = CFGS[li]
                p, mf, n, st, ost = cfg.p, cfg.mf, cfg.n, cfg.st, cfg.ost
                ipl = 16 * st                          # input q-plane size
                nbt = cfg.nbk_out
                s_in = None if li == 0 else s_t[(bp, (li - 1) % 2)]
                s_out = s_t[(bp, li % 2)]
                sov = s_out[:, 0:nbt * T * ost].rearrange(
                    "p (q t w) -> p q t w", t=T, w=ost)
                vth_ap = vthp_t[0:p, li:li + 1]
                c_ap = cc_t[0:p, li:li + 1]
                vv = w_t[bp][:, :].rearrange(
                    "p (k w) -> p k w", w=64)[0:p, 0:nbt, 0:ost]
                uu = u_t[bp][:, :].rearrange(
                    "p (k w) -> p k w", w=64)[0:p, 0:nbt, 0:ost]

                for h in range(2):
                    base_h = h * 8 * st
                    et = epool.tile([128, e_size], F32, tag="e",
                                    name=f"e{b}l{li}h{h}")
                    ev4 = et[0:p, 0:nbt * 8 * ost].rearrange(
                        "p (k t w) -> p k t w", t=8, w=ost)
                    for gi, qs in enumerate(cfg.groups):
                        nbkg = len(qs)
                        ps = ppool.tile([128, 2048], F32, tag="ps",
                                        name=f"ps{b}l{li}h{h}g{gi}")
                        ps_f = ps[:, :]
                        nmm = {}
                        for bi, q in enumerate(qs):
                            _, r, r1, r2 = cfg.banks[q]
                            na = NP1 if li == 0 else cfg.nsp * 4
                            nb = (NP1 if li == 0 else cfg.nsp * 2) \
                                if r2 > 0 else 0
                            nmm[bi] = [na + nb, 0]

                        def mm(bi, lhs, rhs):
                            tot, done = nmm[bi]
                            out_ap = ps_f[0:p, bi * 512: bi * 512 + n]
                            nc.tensor.matmul(out_ap, lhs, rhs,
                                             start=(done == 0),
                                             stop=(done == tot - 1))
                            nmm[bi][1] += 1

                        if li == 0:
                            for pi, (wi, xi) in enumerate(L1_PAIRS):
                                lhs = wa_t[0][0:96, pi * 128:(pi + 1) * 128]
                                xt = x_t[xi][:, :]
                                for bi, q in enumerate(qs):
                                    rhs = xt[0:96, q * 1024 + base_h:
                                             q * 1024 + base_h + n]
                                    mm(bi, lhs, rhs)
                            for pi, (wi, xi) in enumerate(L1_PAIRS):
                                lhs = wb_t[0][0:36, pi * 128:(pi + 1) * 128]
                                xt = x_t[xi][:, :]
                                for bi, q in enumerate(qs):
                                    if cfg.banks[q][3] > 0:
                                        rhs = xt[0:36,
                                                 (q + 1) * 1024 + base_h:
                                                 (q + 1) * 1024 + base_h + n]
                                        mm(bi, lhs, rhs)
                        else:
                            s_in_f = s_in[:, :]
                            for sp in range(cfg.nsp):
                                for dj in range(4):
                                    c0 = (sp * 4 + dj) * mf
                                    for bi, q in enumerate(qs):
                                        k1 = cfg.banks[q][2] * 16
                                        lhs = wa_t[li][0:k1, c0:c0 + mf]
                                        rhs = s_in_f[0:k1,
                                                     q * ipl + base_h + dj:
                                                     q * ipl + base_h + dj + n]
                                        mm(bi, lhs, rhs)
                            for sp in range(cfg.nsp):
                                for g in range(2):
                                    c0 = (sp * 2 + g) * mf
                                    ft = f_t[(bp, g)][:, :]
                                    for bi, q in enumerate(qs):
                                        r2 = cfg.banks[q][3]
                                        if r2 > 0:
                                            wbt = wb_t[li] if r2 == 3 \
                                                else wbp_t[li]
                                            lhs = wbt[0:96, c0:c0 + mf]
                                            rhs = ft[0:96,
                                                     q * ipl + base_h:
                                                     q * ipl + base_h + n]
                                            mm(bi, lhs, rhs)

                        # --- evict psum to SBUF on the ACT engine, negated
                        # with the conv(ones) constant folded: e = -(q+C)
                        # = -dv. Width = ost (even, covers wout). ---
                        ps_v = ps_f.rearrange("p (k f) -> p k f", k=4)
                        ps4 = ps_v[0:p, 0:nbkg, 0:8 * st].rearrange(
                            "p k (t w) -> p k t w", w=st)[:, :, :, 0:ost]
                        epart = ev4[0:p, qs[0]:qs[0] + nbkg, :, :]
                        nc.scalar.activation(
                            epart, ps4, mybir.ActivationFunctionType.Identity,
                            bias=c_ap, scale=-1.0)

                    # --- LIF scan (all DVE, f32 state, inverted spikes):
                    #   u  = v - e                (e = -dv)
                    #   s' = u is_lt vth          (tensor_scalar, 2x mode)
                    #   v  = u * s'
                    for t in range(8):
                        tt = h * 8 + t
                        ev = ev4[0:p, 0:nbt, t, 0:ost]
                        sw = sov[0:p, 0:nbt, tt, 0:ost]
                        if tt == 0:
                            nc.vector.tensor_scalar(
                                uu, ev, -1.0, None, mybir.AluOpType.mult)
                        else:
                            nc.vector.tensor_tensor(
                                out=uu, in0=vv, in1=ev,
                                op=mybir.AluOpType.subtract)
                        nc.vector.tensor_scalar(
                            sw, uu, vth_ap, None, mybir.AluOpType.is_lt)
                        nc.vector.tensor_tensor(
                            out=vv, in0=uu, in1=sw,
                            op=mybir.AluOpType.mult)

            def emit_sout(b):
                nc.gpsimd.dma_start(
                    out=sout_d[b],
                    in_=s_t[(b % 2, 0)][0:48, 0:SOUT_FREE])

            emit_x_dma(0)
            for b, li in SLOT_SEQ:
                emit_layer(b, li)
                if li < 4:
                    emit_spill(b, li + 1)
                else:
                    emit_sout(b)
                if li == 0 and b + 1 < B_LOC:
                    emit_x_dma(b + 1)

    nc.compile()
    _PROGRAM_CACHE["nc"] = nc
    return nc


def _host_inputs(inputs):
    m = _pack_weights(inputs)
    s1, s2 = _pack_scalars(inputs)
    m["vthp"] = s1
    m["cc"] = s2
    return m


def decode_sout(sout):
    """[B_LOC, 48, SOUT_FREE] bf16 -> [B_LOC, T, 6] spike means."""
    a = 1.0 - np.asarray(sout, np.float32).reshape(B_LOC, 8, 6, L5.nbk_out,
                                                   T, L5.ost)
    rho = np.arange(8)[:, None]
    qq = np.arange(L5.nbk_out)[None, :]
    mask = (8 * qq + rho) < L5.hout                     # [rho, q]
    a = a.transpose(0, 4, 2, 1, 3, 5)                   # [b, t, c, rho, q, j]
    vals = a[:, :, :, mask, :][:, :, :, :, :L5.wout]    # [b, t, c, 49, 49]
    return vals.mean(axis=(3, 4)).astype(np.float32)


def run_spmd(inputs, **kw):
    nc = _build_program()
    x = np.asarray(inputs["x"], np.float32)
    const = _host_inputs(inputs)
    in_maps = []
    for c in range(N_CORES):
        m = dict(const)
        m["xr"] = _arrange_x(x[c * B_LOC:(c + 1) * B_LOC])
        in_maps.append(m)
    return run_bass_kernel_spmd(nc, in_maps, list(range(N_CORES)), **kw)


def kernel(**inputs):
    res = run_spmd(inputs)
    outs = [decode_sout(r["sout"]) for r in res.results]
    return np.concatenate(outs, axis=0)
